# revision 37
# baseline (speedup 1.0000x reference)
"""Trainium2 Bass kernel for nn_MoE_790273983069 (moe_routing).

Strategy: data-parallel over the batch across 8 NeuronCores (1024 tokens per
core, all expert weights read by every core, no cross-core communication).
Inside each core: fp32 gating (exact top-4-of-8 via DVE max8), on-device
dispatch (free-axis cumsum + GPSIMD local_scatter builds per-expert slot
tables at fixed capacity), DMA-transpose token gather, bf16 expert MLP on
the TensorEngine, fused exp(out + ln(gate)) eviction, fp32 DMA scatter-add
combine, and a final log pass.

kernel(**inputs) takes the FULL unsharded inputs and returns (y, loss),
matching reference.reference().
"""

import sys

sys.path.insert(0, "/opt/trn_rl_repo")

from contextlib import ExitStack
from dataclasses import dataclass

import ml_dtypes
import numpy as np

import concourse.bass as bass
import concourse.mybir as mybir
from concourse import bacc
from concourse.bass_utils import run_bass_kernel_spmd
from concourse.masks import make_identity
from concourse.tile import TileContext

FP32 = mybir.dt.float32
BF16 = mybir.dt.bfloat16
I16 = mybir.dt.int16
U16 = mybir.dt.uint16
U32 = mybir.dt.uint32
AF = mybir.ActivationFunctionType
ALU = mybir.AluOpType

N_CORES = 8
EPS_COMBINE = float(np.finfo(np.float64).eps)


@dataclass
class Cfg:
    BC: int = 1024   # tokens per core
    D: int = 1024    # d_in
    H: int = 2048    # hidden
    DO: int = 1024   # d_out
    E: int = 8       # experts
    K: int = 4       # top-k
    CAP: int = 640   # per-expert slot capacity (multiple of 128)
    NT: int = 10     # num task ids
    TPAD: int = 64   # padded task-table row (fp32 -> 256B)

    @property
    def FT(self):
        return self.D // 128

    @property
    def HT(self):
        return self.H // 128

    @property
    def TT(self):
        return self.BC // 128

    @property
    def J(self):
        return self.CAP // 128

    @property
    def C16(self):
        return self.CAP // 16


def _chunks(total, step):
    out = []
    o = 0
    while o < total:
        n = min(step, total - o)
        out.append((o, n))
        o += n
    return out


def build_moe(tc, outs, ins, cfg: Cfg):
    """Per-core MoE kernel. ins/outs are dicts of DRAM APs."""
    nc = tc.nc
    ctx = ExitStack()

    xt_d = ins["xt"]          # [D+16, BC] fp32 (x^T with one-hot task rows)
    xb_d = ins["xb"]          # [BC, D] bf16 (row-major, gather source)
    wgx_d = ins["wgx"]        # [D, E] fp32
    tab_d = ins["tab16"]      # [16, E] fp32 (task logit table, padded rows)
    w1_d = ins["w1"]          # [E, HT, FT, 128, 128] bf16 (tiled fc1)
    w2_d = ins["w2"]          # [E, H, DO] bf16
    y_d = outs["y"]           # [BC, DO] fp32
    stats_d = outs["stats"]   # [E, 2] fp32 (importance, load)


    ids_s_h = nc.dram_tensor("ids_s_h", [cfg.E, cfg.CAP], I16).ap()   # slot ids (pad->-1)
    lng_h = nc.dram_tensor("lng_h", [cfg.E, cfg.CAP], FP32).ap()      # ln(gate) per slot

    const = ctx.enter_context(tc.tile_pool(name="const", bufs=1))
    meta = ctx.enter_context(tc.tile_pool(name="meta", bufs=1))
    wp = ctx.enter_context(tc.tile_pool(name="wpool", bufs=3))

    def issue_w1(e):
        tiles = []
        for m in range(cfg.HT):
            w1t = wp.tile([128, cfg.FT, 128], BF16, tag="w1", bufs=24)
            nc.scalar.dma_start(out=w1t[:], in_=w1_d[e, m])
            tiles.append(w1t)
        return tiles

    w1_next = issue_w1(0)

    ident = const.tile([128, 128], FP32)
    make_identity(nc, ident[:])
    epsb = const.tile([128, 1], FP32)
    nc.vector.memset(epsb[:], EPS_COMBINE)
    # SBUF combine accumulators: even/odd token-tiles (parity-split CCE
    # scatter-add destinations)
    y_ev = meta.tile([128, cfg.TT // 2, cfg.DO], FP32)
    y_od = meta.tile([128, cfg.TT // 2, cfg.DO], FP32)
    nc.vector.memset(y_ev[:], 0.0)
    nc.vector.memset(y_od[:], 0.0)

    # per-expert dispatch metadata (filled during gating)
    idxs_sb = [meta.tile([128, cfg.C16], I16, name=f"idxs_sb{e}")
               for e in range(cfg.E)]
    lngb_sb = meta.tile([128, cfg.E, cfg.J], FP32)
    cnt_i32 = meta.tile([1, cfg.E], mybir.dt.int32)

    # ---------------- gating phase ----------------
    with tc.tile_pool(name="gat", bufs=1) as gp, \
         tc.tile_pool(name="gat2", bufs=2) as gp2, \
         tc.tile_pool(name="gpsum", bufs=2, space="PSUM") as gps:

        iot = gp.tile([16, cfg.BC], I16)
        nc.gpsimd.iota(iot[:], pattern=[[1, cfg.BC]], base=1, channel_multiplier=0)

        xt_sb = gp.tile([128, cfg.FT, cfg.BC], FP32)
        xt_src = xt_d[0:cfg.D].rearrange("(f p) t -> p f t", p=128)
        for f in range(cfg.FT):
            nc.sync.dma_start(out=xt_sb[:, f, :], in_=xt_src[:, f, :])
        xtau_sb = gp.tile([16, cfg.BC], FP32)
        nc.sync.dma_start(out=xtau_sb[:], in_=xt_d[cfg.D:cfg.D + 16, :])
        wgx_sb = gp.tile([128, cfg.FT, cfg.E], FP32)
        nc.sync.dma_start(out=wgx_sb[:], in_=wgx_d.rearrange("(f p) e -> p f e", p=128))
        tab_sb = gp.tile([16, cfg.E], FP32)
        nc.sync.dma_start(out=tab_sb[:], in_=tab_d[:, :])

        maskT = meta.tile([16, cfg.BC], FP32)
        lngT = meta.tile([16, cfg.BC], FP32)
        maskT_m = maskT
        nc.vector.memset(maskT[:], 0.0)
        nc.vector.memset(lngT[:], 0.0)

        # pass 1a: token-major logits (task table folded in via one-hot rows)
        logits_all = meta.tile([128, cfg.TT, cfg.E], FP32)
        m8_all = meta.tile([128, cfg.TT, 8], FP32)
        for i in range(cfg.TT):
            lg_ps = gps.tile([128, cfg.E], FP32, tag="lg")
            for f in range(cfg.FT):
                nc.tensor.matmul(
                    lg_ps[:],
                    xt_sb[:, f, i * 128:(i + 1) * 128],
                    wgx_sb[:, f, :],
                    start=(f == 0), stop=False,
                )
            nc.tensor.matmul(
                lg_ps[:],
                xtau_sb[:, i * 128:(i + 1) * 128],
                tab_sb[:],
                start=False, stop=True,
            )
            nc.vector.tensor_copy(logits_all[:, i, :], lg_ps[:])

        # pass 1b: per-tile top-8 + mask + expert-major mask tiles
        for i in range(cfg.TT):
            nc.vector.max(out=m8_all[:, i, :], in_=logits_all[:, i, :])
            mask = gp2.tile([128, cfg.E], FP32, tag="mask")
            nc.vector.tensor_scalar(
                out=mask[:], in0=logits_all[:, i, :],
                scalar1=m8_all[:, i, cfg.K - 1:cfg.K],
                scalar2=None, op0=ALU.is_ge,
            )
            mt_ps = gps.tile([cfg.E, 128], FP32, tag="mt")
            nc.tensor.transpose(mt_ps[:], mask[:], ident[:])
            nc.vector.tensor_copy(maskT[0:cfg.E, i * 128:(i + 1) * 128], mt_ps[:])

        # ---- gather-critical ids chain ----
        zeros16 = gp.tile([16, cfg.BC], FP32)
        nc.vector.memset(zeros16[:], 0.0)
        cum = gp.tile([16, cfg.BC], FP32)
        nc.vector.tensor_tensor_scan(
            out=cum[:], data0=maskT[:], data1=zeros16[:], initial=0.0,
            op0=ALU.add, op1=ALU.add,
        )
        fits = gp.tile([16, cfg.BC], FP32)
        nc.vector.tensor_scalar(out=fits[:], in0=cum[:], scalar1=float(cfg.CAP),
                                scalar2=None, op0=ALU.is_le)
        mask2 = gp.tile([16, cfg.BC], FP32)
        nc.vector.tensor_tensor(out=mask2[:], in0=maskT[:], in1=fits[:], op=ALU.mult)
        idxf = gp.tile([16, cfg.BC], FP32)
        nc.vector.tensor_tensor(out=idxf[:], in0=cum[:], in1=mask2[:], op=ALU.mult)
        nc.vector.tensor_scalar(out=idxf[:], in0=idxf[:], scalar1=1.0,
                                scalar2=None, op0=ALU.subtract)
        idxi = meta.tile([16, cfg.BC], I16)
        nc.vector.tensor_copy(idxi[:], idxf[:])
        slots1 = gp.tile([16, cfg.CAP], I16)
        nc.gpsimd.local_scatter(
            out_ap=slots1[:], data_ap=iot[:], idxs_ap=idxi[:],
            channels=16, num_elems=cfg.CAP, num_idxs=cfg.BC,
        )
        ids_s = gp.tile([16, cfg.CAP], I16)
        nc.vector.tensor_scalar(out=ids_s[:], in0=slots1[:], scalar1=1,
                                scalar2=None, op0=ALU.subtract)

        loadT = meta.tile([16, 1], FP32)
        loadT_m = loadT
        nc.vector.tensor_reduce(out=loadT[:], in_=maskT[:],
                                axis=mybir.AxisListType.X, op=ALU.add)
        cnt_col = gp.tile([cfg.E, 1], FP32)
        nc.vector.tensor_scalar(out=cnt_col[:], in0=loadT[0:cfg.E, :],
                                scalar1=float(cfg.CAP), scalar2=None, op0=ALU.min)
        cr_ps = gps.tile([1, cfg.E], FP32, tag="mt")
        nc.tensor.transpose(cr_ps[:], cnt_col[:], ident[0:cfg.E, 0:cfg.E])
        nc.vector.tensor_copy(cnt_i32[:], cr_ps[:])

        nc.sync.dma_start(out=ids_s_h[:, :], in_=ids_s[0:cfg.E, :])
        for e in range(cfg.E):
            srcs = ids_s_h[e:e + 1, :].rearrange("o (c s) -> (o s) c", s=16)
            nc.sync.dma_start(out=idxs_sb[e][0:16, :], in_=srcs)
            for lo, n in ((16, 16), (32, 32), (64, 64)):
                nc.sync.dma_start(out=idxs_sb[e][lo:lo + n, :], in_=idxs_sb[e][0:n, :])

    # ---------------- expert phase ----------------
    n_chunks = _chunks(cfg.CAP, 512)
    o_chunks = _chunks(cfg.DO, 512)

    cnt_regs = [
        nc.values_load(cnt_i32[0:1, e:e + 1], engines=[mybir.EngineType.Pool],
                       min_val=0, max_val=cfg.CAP, skip_runtime_bounds_check=True)
        for e in range(cfg.E)
    ]

    with tc.tile_pool(name="apool", bufs=2) as ap_, \
         tc.tile_pool(name="ps1", bufs=2, space="PSUM") as ps1, \
         tc.tile_pool(name="ps2", bufs=2, space="PSUM") as ps2:

        def issue_gather(e):
            xg = ap_.tile([128, cfg.FT, cfg.CAP], BF16, tag="xg")
            nc.gpsimd.dma_gather(
                out_ap=xg[:],
                in_ap=xb_d[:, :],
                idxs_ap=idxs_sb[e][:],
                num_idxs=cfg.CAP,
                num_idxs_reg=cnt_regs[e],
                elem_size=cfg.D,
                transpose=True,
            )
            return xg

        xg_next = issue_gather(0)

        # ---- softmax / ln-gate chain (traced after weight-load issues) ----
        e4_all = meta.tile([128, cfg.TT, cfg.K], FP32)
        nc.scalar.activation(e4_all[:], m8_all[:, :, 0:cfg.K], AF.Exp)
        den_all = meta.tile([128, cfg.TT], FP32)
        nc.vector.tensor_reduce(out=den_all[:], in_=e4_all[:],
                                axis=mybir.AxisListType.X, op=ALU.add)
        lnden_all = meta.tile([128, cfg.TT], FP32)
        nc.scalar.activation(lnden_all[:], den_all[:], AF.Ln)
        lng_all = meta.tile([128, cfg.TT, cfg.E], FP32)
        for i in range(cfg.TT):
            nc.vector.tensor_scalar(
                out=lng_all[:, i, :], in0=logits_all[:, i, :],
                scalar1=lnden_all[:, i:i + 1],
                scalar2=None, op0=ALU.subtract,
            )
            lt_ps = ps1.tile([cfg.E, 128], FP32, tag="lt")
            nc.tensor.transpose(lt_ps[:], lng_all[:, i, :], ident[:])
            nc.vector.tensor_copy(lngT[0:cfg.E, i * 128:(i + 1) * 128], lt_ps[:])


        # ln(gate) per slot, scattered as two 16-bit halves (off the
        # gather-critical Pool path: traced after the first gather)
        lp = meta
        lngT16 = lngT[:].bitcast(U16).rearrange("p (t two) -> p t two", two=2)
        lo_c = lp.tile([16, cfg.BC], U16)
        nc.vector.tensor_copy(lo_c[:], lngT16[:, :, 0])
        hi_c = lp.tile([16, cfg.BC], U16)
        nc.vector.tensor_copy(hi_c[:], lngT16[:, :, 1])
        lo_s = lp.tile([16, cfg.CAP], U16)
        nc.gpsimd.local_scatter(out_ap=lo_s[:], data_ap=lo_c[:], idxs_ap=idxi[:],
                                channels=16, num_elems=cfg.CAP, num_idxs=cfg.BC)
        hi_s = lp.tile([16, cfg.CAP], U16)
        nc.gpsimd.local_scatter(out_ap=hi_s[:], data_ap=hi_c[:], idxs_ap=idxi[:],
                                channels=16, num_elems=cfg.CAP, num_idxs=cfg.BC)
        lo_u = lp.tile([16, cfg.CAP], U32)
        nc.vector.tensor_copy(lo_u[:], lo_s[:])
        hi_u = lp.tile([16, cfg.CAP], U32)
        nc.vector.tensor_copy(hi_u[:], hi_s[:])
        nc.vector.tensor_scalar(out=hi_u[:], in0=hi_u[:], scalar1=16,
                                scalar2=None, op0=ALU.logical_shift_left)
        lng_u = lp.tile([16, cfg.CAP], U32)
        nc.vector.tensor_tensor(out=lng_u[:], in0=hi_u[:], in1=lo_u[:],
                                op=ALU.bitwise_or)
        nc.sync.dma_start(out=lng_h[:, :], in_=lng_u[0:cfg.E, :].bitcast(FP32))
        for e in range(cfg.E):
            nc.sync.dma_start(
                out=lngb_sb[:, e, :],
                in_=lng_h[e:e + 1, :].rearrange("o (j p) -> (o p) j", p=128),
            )

        def emit_stats():
            gexpT = meta.tile([16, cfg.BC], FP32)
            nc.scalar.activation(gexpT[:], lngT[:], AF.Exp)
            gatesT = meta.tile([16, cfg.BC], FP32)
            nc.vector.tensor_tensor(out=gatesT[:], in0=gexpT[:], in1=maskT_m[:],
                                    op=ALU.mult)
            impT = meta.tile([16, 1], FP32)
            nc.vector.tensor_reduce(out=impT[:], in_=gatesT[:],
                                    axis=mybir.AxisListType.X, op=ALU.add)
            st_sb = meta.tile([cfg.E, 2], FP32)
            nc.vector.tensor_copy(st_sb[:, 0:1], impT[0:cfg.E, :])
            nc.vector.tensor_copy(st_sb[:, 1:2], loadT_m[0:cfg.E, :])
            nc.sync.dma_start(out=stats_d[:, :], in_=st_sb[:])

        for e in range(cfg.E):
            xg = xg_next
            w1_tiles = w1_next
            if e == cfg.E - 2:
                emit_stats()
            if e + 1 < cfg.E:
                xg_next = issue_gather(e + 1)
                w1_next = issue_w1(e + 1)

            hT = ap_.tile([128, cfg.HT, cfg.CAP], BF16, tag="hT", bufs=1)
            w2sb = wp.tile([128, cfg.HT, cfg.DO], BF16, tag="w2", bufs=1)
            w2src = w2_d[e].rearrange("(h p) o -> p h o", p=128)
            for m in range(cfg.HT):
                w1t = w1_tiles[m]
                for (no, nn) in n_chunks:
                    h_ps = ps1.tile([128, min(512, cfg.CAP)], FP32, tag="h")
                    for f in range(cfg.FT):
                        nc.tensor.matmul(
                            h_ps[:, 0:nn],
                            w1t[:, f, :],
                            xg[:, f, no:no + nn],
                            start=(f == 0),
                            stop=(f == cfg.FT - 1),
                        )
                    nc.scalar.activation(hT[:, m, no:no + nn], h_ps[:, 0:nn], AF.Relu)
                nc.sync.dma_start(out=w2sb[:, m, :], in_=w2src[:, m, :])

            eo = ap_.tile([128, cfg.J, cfg.DO], FP32, tag="eo", bufs=1)
            for j in range(cfg.J):
                o_ps = ps2.tile([128, cfg.DO], FP32, tag="o")
                for h in range(cfg.HT):
                    for (oo, on) in o_chunks:
                        nc.tensor.matmul(
                            o_ps[:, oo:oo + on],
                            hT[:, h, j * 128:(j + 1) * 128],
                            w2sb[:, h, oo:oo + on],
                            start=(h == 0),
                            stop=(h == cfg.HT - 1),
                        )
                nc.scalar.activation(eo[:, j, :], o_ps[:], AF.Exp,
                                     bias=lngb_sb[:, e, j:j + 1])
            if e < cfg.E - 1:
                nc.gpsimd.dma_scatter_add(
                    out_ap=y_ev[:],
                    out_ap_other=y_od[:],
                    in_ap=eo[:],
                    idxs_ap=idxs_sb[e][:],
                    num_idxs=cfg.CAP,
                    num_idxs_reg=cnt_regs[e],
                    elem_size=cfg.DO,
                    sbuf_tokens_per_rank=128,
                    parity_reg=0,
                )
            else:
                # split the last expert's combine per 128-slot tile so the
                # finalize can begin as soon as the final sliver lands
                from concourse.expressions import smax, smin
                for j in range(cfg.J):
                    r_j = smax(smin(cnt_regs[e] - 128 * j, 128), 0)
                    nc.gpsimd.dma_scatter_add(
                        out_ap=y_ev[:],
                        out_ap_other=y_od[:],
                        in_ap=eo[:, j:j + 1, :],
                        idxs_ap=idxs_sb[e][:, j * 8:(j + 1) * 8],
                        num_idxs=128,
                        num_idxs_reg=r_j,
                        elem_size=cfg.DO,
                        sbuf_tokens_per_rank=128,
                        parity_reg=0,
                    )

    # ---------------- finalize: y = log(max(acc, eps-fix)) ----------------
    with tc.tile_pool(name="fin", bufs=3) as fp:
        for i in range(cfg.TT):
            acc = (y_ev if i % 2 == 0 else y_od)[:, i // 2, :]
            yo = fp.tile([128, cfg.DO], FP32, tag="yo")
            nc.scalar.activation(yo[:], acc, AF.Ln, bias=epsb[:])
            nc.sync.dma_start(out=y_d[i * 128:(i + 1) * 128, :], in_=yo[:])

    ctx.close()


# ---------------------------------------------------------------------------
# host side
# ---------------------------------------------------------------------------

_COMPILED = {}


def _get_compiled(cfg: Cfg):
    key = tuple(sorted(cfg.__dict__.items()))
    if key in _COMPILED:
        return _COMPILED[key]
    nc = bacc.Bacc("TRN2", target_bir_lowering=False, debug=False,
                   num_devices=N_CORES, num_swdge_queues=2)
    ins = {
        "xt": nc.dram_tensor("xt", [cfg.D + 16, cfg.BC], FP32, kind="ExternalInput").ap(),
        "xb": nc.dram_tensor("xb", [cfg.BC, cfg.D], BF16, kind="ExternalInput").ap(),
        "wgx": nc.dram_tensor("wgx", [cfg.D, cfg.E], FP32, kind="ExternalInput").ap(),
        "tab16": nc.dram_tensor("tab16", [16, cfg.E], FP32, kind="ExternalInput").ap(),
        "w1": nc.dram_tensor("w1", [cfg.E, cfg.HT, 128, cfg.FT, 128], BF16, kind="ExternalInput").ap(),
        "w2": nc.dram_tensor("w2", [cfg.E, cfg.H, cfg.DO], BF16, kind="ExternalInput").ap(),
    }
    outs = {
        "y": nc.dram_tensor("y", [cfg.BC, cfg.DO], FP32, kind="ExternalOutput").ap(),
        "stats": nc.dram_tensor("stats", [cfg.E, 2], FP32, kind="ExternalOutput").ap(),
    }
    with TileContext(nc) as tc:
        build_moe(tc, outs, ins, cfg)
    nc.compile()
    _COMPILED[key] = nc
    return nc


def _wrap16(ids: np.ndarray) -> np.ndarray:
    """[N] -> [128, N//16] int16, 16-wrapped and replicated across 8 cores."""
    n = ids.shape[0]
    w = ids.reshape(n // 16, 16).T.astype(np.int16)
    return np.tile(w, (8, 1))


def prepare_inputs(x, task_ids, task_matrix, task_W, task_b, w_gate,
                   fc1_w, fc1_b, fc2_w, fc2_b, cfg: Cfg):
    """Shard + lay out inputs for the 8 cores. Pure data marshalling on host
    (plus folding the tiny task-embedding linear algebra into a 10x8 table)."""
    D = cfg.D
    assert np.allclose(fc1_b, 0.0) and np.allclose(fc2_b, 0.0), \
        "kernel assumes zero expert biases (as produced by setup_inputs)"

    # task logit table: Linear(task_emb) @ w_gate[D:]  ->  [16, E] (padded)
    wg_t = w_gate[D:].astype(np.float64)
    tab = (task_matrix.astype(np.float64) @ task_W.astype(np.float64) @ wg_t
           + task_b.astype(np.float64) @ wg_t).astype(np.float32)
    tab16 = np.zeros((16, cfg.E), np.float32)
    tab16[:cfg.NT] = tab

    wgx = np.ascontiguousarray(w_gate[:D], np.float32)
    w1t = np.ascontiguousarray(
        fc1_w.astype(ml_dtypes.bfloat16)
        .reshape(cfg.E, cfg.FT, 128, cfg.HT, 128)
        .transpose(0, 3, 2, 1, 4)
    )
    w2b = np.ascontiguousarray(fc2_w.astype(ml_dtypes.bfloat16))

    in_maps = []
    for c in range(N_CORES):
        xs = x[c * cfg.BC:(c + 1) * cfg.BC]
        ts = task_ids[c * cfg.BC:(c + 1) * cfg.BC]
        onehot = np.zeros((16, cfg.BC), np.float32)
        onehot[ts, np.arange(cfg.BC)] = 1.0
        in_maps.append({
            "xt": np.ascontiguousarray(
                np.concatenate([xs.T.astype(np.float32), onehot], axis=0)),
            "xb": np.ascontiguousarray(xs.astype(ml_dtypes.bfloat16)),
            "wgx": wgx,
            "tab16": tab16,
            "w1": w1t,
            "w2": w2b,
        })
    return in_maps


def _cv_squared(v: np.ndarray) -> np.float32:
    eps = np.float32(1e-10)
    v = v.astype(np.float32)
    return np.var(v, ddof=1, dtype=np.float32) / (np.mean(v, dtype=np.float32) ** 2 + eps)


def kernel(x, task_ids, task_matrix, task_W, task_b, w_gate,
           fc1_w, fc1_b, fc2_w, fc2_b, **run_kwargs):
    cfg = Cfg()
    x = np.asarray(x, np.float32)
    task_ids = np.asarray(task_ids, np.int32)
    in_maps = prepare_inputs(
        x, task_ids, np.asarray(task_matrix, np.float32),
        np.asarray(task_W, np.float32), np.asarray(task_b, np.float32),
        np.asarray(w_gate, np.float32), np.asarray(fc1_w, np.float32),
        np.asarray(fc1_b, np.float32), np.asarray(fc2_w, np.float32),
        np.asarray(fc2_b, np.float32), cfg)
    nc = _get_compiled(cfg)
    res = run_bass_kernel_spmd(nc, in_maps, list(range(N_CORES)), **run_kwargs)
    y = np.concatenate([res.results[c]["y"] for c in range(N_CORES)], axis=0)
    stats = np.stack([res.results[c]["stats"] for c in range(N_CORES)])
    importance = stats[:, :, 0].sum(0)
    load = stats[:, :, 1].sum(0)
    loss = np.float32(_cv_squared(importance) + _cv_squared(load))
    kernel.last_results = res
    return y, loss


# revision 38
# speedup vs baseline: 1.0063x; 1.0063x over previous
"""Trainium2 Bass kernel for nn_MoE_790273983069 (moe_routing).

Strategy: data-parallel over the batch across 8 NeuronCores (1024 tokens per
core, all expert weights read by every core, no cross-core communication).
Inside each core: fp32 gating (exact top-4-of-8 via DVE max8), on-device
dispatch (free-axis cumsum + GPSIMD local_scatter builds per-expert slot
tables at fixed capacity), DMA-transpose token gather, bf16 expert MLP on
the TensorEngine, fused exp(out + ln(gate)) eviction, fp32 DMA scatter-add
combine, and a final log pass.

kernel(**inputs) takes the FULL unsharded inputs and returns (y, loss),
matching reference.reference().
"""

import sys

sys.path.insert(0, "/opt/trn_rl_repo")

from contextlib import ExitStack
from dataclasses import dataclass

import ml_dtypes
import numpy as np

import concourse.bass as bass
import concourse.mybir as mybir
from concourse import bacc
from concourse.bass_utils import run_bass_kernel_spmd
from concourse.masks import make_identity
from concourse.tile import TileContext

FP32 = mybir.dt.float32
BF16 = mybir.dt.bfloat16
I16 = mybir.dt.int16
U16 = mybir.dt.uint16
U32 = mybir.dt.uint32
AF = mybir.ActivationFunctionType
ALU = mybir.AluOpType

N_CORES = 8
EPS_COMBINE = float(np.finfo(np.float64).eps)


@dataclass
class Cfg:
    BC: int = 1024   # tokens per core
    D: int = 1024    # d_in
    H: int = 2048    # hidden
    DO: int = 1024   # d_out
    E: int = 8       # experts
    K: int = 4       # top-k
    CAP: int = 640   # per-expert slot capacity (multiple of 128)
    NT: int = 10     # num task ids
    TPAD: int = 64   # padded task-table row (fp32 -> 256B)

    @property
    def FT(self):
        return self.D // 128

    @property
    def HT(self):
        return self.H // 128

    @property
    def TT(self):
        return self.BC // 128

    @property
    def J(self):
        return self.CAP // 128

    @property
    def C16(self):
        return self.CAP // 16


def _chunks(total, step):
    out = []
    o = 0
    while o < total:
        n = min(step, total - o)
        out.append((o, n))
        o += n
    return out


def build_moe(tc, outs, ins, cfg: Cfg):
    """Per-core MoE kernel. ins/outs are dicts of DRAM APs."""
    nc = tc.nc
    ctx = ExitStack()

    xt_d = ins["xt"]          # [D+16, BC] fp32 (x^T with one-hot task rows)
    xb_d = ins["xb"]          # [BC, D] bf16 (row-major, gather source)
    wgx_d = ins["wgx"]        # [D, E] fp32
    tab_d = ins["tab16"]      # [16, E] fp32 (task logit table, padded rows)
    w1_d = ins["w1"]          # [E, HT, FT, 128, 128] bf16 (tiled fc1)
    w2_d = ins["w2"]          # [E, H, DO] bf16
    y_d = outs["y"]           # [BC, DO] fp32
    stats_d = outs["stats"]   # [E, 2] fp32 (importance, load)


    ids_s_h = nc.dram_tensor("ids_s_h", [cfg.E, cfg.CAP], I16).ap()   # slot ids (pad->-1)
    lng_h = nc.dram_tensor("lng_h", [cfg.E, cfg.CAP], FP32).ap()      # ln(gate) per slot

    const = ctx.enter_context(tc.tile_pool(name="const", bufs=1))
    meta = ctx.enter_context(tc.tile_pool(name="meta", bufs=1))
    wp = ctx.enter_context(tc.tile_pool(name="wpool", bufs=3))

    def issue_w1(e):
        tiles = []
        for m in range(cfg.HT):
            w1t = wp.tile([128, cfg.FT, 128], BF16, tag="w1", bufs=24)
            nc.scalar.dma_start(out=w1t[:], in_=w1_d[e, m])
            tiles.append(w1t)
        return tiles

    w1_next = issue_w1(0)

    ident = const.tile([128, 128], FP32)
    make_identity(nc, ident[:])
    epsb = const.tile([128, 1], FP32)
    nc.vector.memset(epsb[:], EPS_COMBINE)
    # SBUF combine accumulators: even/odd token-tiles (parity-split CCE
    # scatter-add destinations)
    y_ev = meta.tile([128, cfg.TT // 2, cfg.DO], FP32)
    y_od = meta.tile([128, cfg.TT // 2, cfg.DO], FP32)
    nc.vector.memset(y_ev[:], 0.0)
    nc.vector.memset(y_od[:], 0.0)

    # per-expert dispatch metadata (filled during gating)
    idxs_sb = [meta.tile([128, cfg.C16], I16, name=f"idxs_sb{e}")
               for e in range(cfg.E)]
    lngb_sb = meta.tile([128, cfg.E, cfg.J], FP32)
    cnt_i32 = meta.tile([1, cfg.E], mybir.dt.int32)

    # ---------------- gating phase ----------------
    with tc.tile_pool(name="gat", bufs=1) as gp, \
         tc.tile_pool(name="gat2", bufs=2) as gp2, \
         tc.tile_pool(name="gpsum", bufs=2, space="PSUM") as gps:

        iot = gp.tile([16, cfg.BC], I16)
        nc.gpsimd.iota(iot[:], pattern=[[1, cfg.BC]], base=1, channel_multiplier=0)

        xt_sb = gp.tile([128, cfg.FT, cfg.BC], FP32)
        xt_src = xt_d[0:cfg.D].rearrange("(f p) t -> p f t", p=128)
        for f in range(cfg.FT):
            nc.sync.dma_start(out=xt_sb[:, f, :], in_=xt_src[:, f, :])
        xtau_sb = gp.tile([16, cfg.BC], FP32)
        nc.sync.dma_start(out=xtau_sb[:], in_=xt_d[cfg.D:cfg.D + 16, :])
        wgx_sb = gp.tile([128, cfg.FT, cfg.E], FP32)
        nc.sync.dma_start(out=wgx_sb[:], in_=wgx_d.rearrange("(f p) e -> p f e", p=128))
        tab_sb = gp.tile([16, cfg.E], FP32)
        nc.sync.dma_start(out=tab_sb[:], in_=tab_d[:, :])

        maskT = meta.tile([16, cfg.BC], FP32)
        lngT = meta.tile([16, cfg.BC], FP32)
        maskT_m = maskT
        nc.vector.memset(maskT[:], 0.0)
        nc.vector.memset(lngT[:], 0.0)

        # pass 1a: token-major logits (task table folded in via one-hot rows)
        logits_all = meta.tile([128, cfg.TT, cfg.E], FP32)
        m8_all = meta.tile([128, cfg.TT, 8], FP32)
        for i in range(cfg.TT):
            lg_ps = gps.tile([128, cfg.E], FP32, tag="lg")
            for f in range(cfg.FT):
                nc.tensor.matmul(
                    lg_ps[:],
                    xt_sb[:, f, i * 128:(i + 1) * 128],
                    wgx_sb[:, f, :],
                    start=(f == 0), stop=False,
                )
            nc.tensor.matmul(
                lg_ps[:],
                xtau_sb[:, i * 128:(i + 1) * 128],
                tab_sb[:],
                start=False, stop=True,
            )
            nc.vector.tensor_copy(logits_all[:, i, :], lg_ps[:])

        # pass 1b: per-tile top-8 + mask + expert-major mask tiles
        for i in range(cfg.TT):
            nc.vector.max(out=m8_all[:, i, :], in_=logits_all[:, i, :])
            mask = gp2.tile([128, cfg.E], FP32, tag="mask")
            nc.vector.tensor_scalar(
                out=mask[:], in0=logits_all[:, i, :],
                scalar1=m8_all[:, i, cfg.K - 1:cfg.K],
                scalar2=None, op0=ALU.is_ge,
            )
            mt_ps = gps.tile([cfg.E, 128], FP32, tag="mt")
            nc.tensor.transpose(mt_ps[:], mask[:], ident[:])
            nc.vector.tensor_copy(maskT[0:cfg.E, i * 128:(i + 1) * 128], mt_ps[:])

        # ---- gather-critical ids chain ----
        zeros16 = gp.tile([16, cfg.BC], FP32)
        nc.vector.memset(zeros16[:], 0.0)
        cum = gp.tile([16, cfg.BC], FP32)
        nc.vector.tensor_tensor_scan(
            out=cum[:], data0=maskT[:], data1=zeros16[:], initial=0.0,
            op0=ALU.add, op1=ALU.add,
        )
        fits = gp.tile([16, cfg.BC], FP32)
        nc.vector.tensor_scalar(out=fits[:], in0=cum[:], scalar1=float(cfg.CAP),
                                scalar2=None, op0=ALU.is_le)
        mask2 = gp.tile([16, cfg.BC], FP32)
        nc.vector.tensor_tensor(out=mask2[:], in0=maskT[:], in1=fits[:], op=ALU.mult)
        idxf = gp.tile([16, cfg.BC], FP32)
        nc.vector.tensor_tensor(out=idxf[:], in0=cum[:], in1=mask2[:], op=ALU.mult)
        nc.vector.tensor_scalar(out=idxf[:], in0=idxf[:], scalar1=1.0,
                                scalar2=None, op0=ALU.subtract)
        idxi = meta.tile([16, cfg.BC], I16)
        nc.vector.tensor_copy(idxi[:], idxf[:])
        slots1 = gp.tile([16, cfg.CAP], I16)
        nc.gpsimd.local_scatter(
            out_ap=slots1[:], data_ap=iot[:], idxs_ap=idxi[:],
            channels=16, num_elems=cfg.CAP, num_idxs=cfg.BC,
        )
        ids_s = gp.tile([16, cfg.CAP], I16)
        nc.vector.tensor_scalar(out=ids_s[:], in0=slots1[:], scalar1=1,
                                scalar2=None, op0=ALU.subtract)

        loadT = meta.tile([16, 1], FP32)
        loadT_m = loadT
        nc.vector.tensor_reduce(out=loadT[:], in_=maskT[:],
                                axis=mybir.AxisListType.X, op=ALU.add)
        cnt_col = gp.tile([cfg.E, 1], FP32)
        nc.vector.tensor_scalar(out=cnt_col[:], in0=loadT[0:cfg.E, :],
                                scalar1=float(cfg.CAP), scalar2=None, op0=ALU.min)
        cr_ps = gps.tile([1, cfg.E], FP32, tag="mt")
        nc.tensor.transpose(cr_ps[:], cnt_col[:], ident[0:cfg.E, 0:cfg.E])
        nc.vector.tensor_copy(cnt_i32[:], cr_ps[:])

        nc.sync.dma_start(out=ids_s_h[:, :], in_=ids_s[0:cfg.E, :])
        for e in range(cfg.E):
            srcs = ids_s_h[e:e + 1, :].rearrange("o (c s) -> (o s) c", s=16)
            nc.sync.dma_start(out=idxs_sb[e][0:16, :], in_=srcs)
            for lo, n in ((16, 16), (32, 32), (64, 64)):
                nc.sync.dma_start(out=idxs_sb[e][lo:lo + n, :], in_=idxs_sb[e][0:n, :])

    # ---------------- expert phase ----------------
    n_chunks = _chunks(cfg.CAP, 512)
    o_chunks = _chunks(cfg.DO, 512)

    cnt_regs = [
        nc.values_load(cnt_i32[0:1, e:e + 1], engines=[mybir.EngineType.Pool],
                       min_val=0, max_val=cfg.CAP, skip_runtime_bounds_check=True)
        for e in range(cfg.E)
    ]

    with tc.tile_pool(name="apool", bufs=2) as ap_, \
         tc.tile_pool(name="ps1", bufs=2, space="PSUM") as ps1, \
         tc.tile_pool(name="ps2", bufs=2, space="PSUM") as ps2:

        def issue_gather(e):
            xg = ap_.tile([128, cfg.FT, cfg.CAP], BF16, tag="xg")
            nc.gpsimd.dma_gather(
                out_ap=xg[:],
                in_ap=xb_d[:, :],
                idxs_ap=idxs_sb[e][:],
                num_idxs=cfg.CAP,
                num_idxs_reg=cnt_regs[e],
                elem_size=cfg.D,
                transpose=True,
            )
            return xg

        xg_next = issue_gather(0)

        # ---- softmax / ln-gate chain (traced after weight-load issues) ----
        e4_all = meta.tile([128, cfg.TT, cfg.K], FP32)
        nc.scalar.activation(e4_all[:], m8_all[:, :, 0:cfg.K], AF.Exp)
        den_all = meta.tile([128, cfg.TT], FP32)
        nc.vector.tensor_reduce(out=den_all[:], in_=e4_all[:],
                                axis=mybir.AxisListType.X, op=ALU.add)
        lnden_all = meta.tile([128, cfg.TT], FP32)
        nc.scalar.activation(lnden_all[:], den_all[:], AF.Ln)
        lng_all = meta.tile([128, cfg.TT, cfg.E], FP32)
        for i in range(cfg.TT):
            nc.vector.tensor_scalar(
                out=lng_all[:, i, :], in0=logits_all[:, i, :],
                scalar1=lnden_all[:, i:i + 1],
                scalar2=None, op0=ALU.subtract,
            )
            lt_ps = ps1.tile([cfg.E, 128], FP32, tag="lt")
            nc.tensor.transpose(lt_ps[:], lng_all[:, i, :], ident[:])
            nc.vector.tensor_copy(lngT[0:cfg.E, i * 128:(i + 1) * 128], lt_ps[:])


        # ln(gate) per slot, scattered as two 16-bit halves (off the
        # gather-critical Pool path: traced after the first gather)
        lp = meta
        lngT16 = lngT[:].bitcast(U16).rearrange("p (t two) -> p t two", two=2)
        lo_c = lp.tile([16, cfg.BC], U16)
        nc.vector.tensor_copy(lo_c[:], lngT16[:, :, 0])
        hi_c = lp.tile([16, cfg.BC], U16)
        nc.vector.tensor_copy(hi_c[:], lngT16[:, :, 1])
        lo_s = lp.tile([16, cfg.CAP], U16)
        nc.gpsimd.local_scatter(out_ap=lo_s[:], data_ap=lo_c[:], idxs_ap=idxi[:],
                                channels=16, num_elems=cfg.CAP, num_idxs=cfg.BC)
        hi_s = lp.tile([16, cfg.CAP], U16)
        nc.gpsimd.local_scatter(out_ap=hi_s[:], data_ap=hi_c[:], idxs_ap=idxi[:],
                                channels=16, num_elems=cfg.CAP, num_idxs=cfg.BC)
        lo_u = lp.tile([16, cfg.CAP], U32)
        nc.vector.tensor_copy(lo_u[:], lo_s[:])
        hi_u = lp.tile([16, cfg.CAP], U32)
        nc.vector.tensor_copy(hi_u[:], hi_s[:])
        nc.vector.tensor_scalar(out=hi_u[:], in0=hi_u[:], scalar1=16,
                                scalar2=None, op0=ALU.logical_shift_left)
        lng_u = lp.tile([16, cfg.CAP], U32)
        nc.vector.tensor_tensor(out=lng_u[:], in0=hi_u[:], in1=lo_u[:],
                                op=ALU.bitwise_or)
        nc.sync.dma_start(out=lng_h[:, :], in_=lng_u[0:cfg.E, :].bitcast(FP32))
        for e in range(cfg.E):
            nc.sync.dma_start(
                out=lngb_sb[:, e, :],
                in_=lng_h[e:e + 1, :].rearrange("o (j p) -> (o p) j", p=128),
            )

        def emit_stats():
            gexpT = meta.tile([16, cfg.BC], FP32)
            nc.scalar.activation(gexpT[:], lngT[:], AF.Exp)
            gatesT = meta.tile([16, cfg.BC], FP32)
            nc.vector.tensor_tensor(out=gatesT[:], in0=gexpT[:], in1=maskT_m[:],
                                    op=ALU.mult)
            impT = meta.tile([16, 1], FP32)
            nc.vector.tensor_reduce(out=impT[:], in_=gatesT[:],
                                    axis=mybir.AxisListType.X, op=ALU.add)
            st_sb = meta.tile([cfg.E, 2], FP32)
            nc.vector.tensor_copy(st_sb[:, 0:1], impT[0:cfg.E, :])
            nc.vector.tensor_copy(st_sb[:, 1:2], loadT_m[0:cfg.E, :])
            nc.sync.dma_start(out=stats_d[:, :], in_=st_sb[:])

        for e in range(cfg.E):
            xg = xg_next
            w1_tiles = w1_next
            if e == cfg.E - 2:
                emit_stats()
            if e + 1 < cfg.E:
                xg_next = issue_gather(e + 1)
                w1_next = []

            hT = ap_.tile([128, cfg.HT, cfg.CAP], BF16, tag="hT", bufs=1)
            w2sb = wp.tile([128, cfg.HT, cfg.DO], BF16, tag="w2", bufs=1)
            w2src = w2_d[e].rearrange("(h p) o -> p h o", p=128)
            for m in range(cfg.HT):
                w1t = w1_tiles[m]
                for (no, nn) in n_chunks:
                    h_ps = ps1.tile([128, min(512, cfg.CAP)], FP32, tag="h")
                    for f in range(cfg.FT):
                        nc.tensor.matmul(
                            h_ps[:, 0:nn],
                            w1t[:, f, :],
                            xg[:, f, no:no + nn],
                            start=(f == 0),
                            stop=(f == cfg.FT - 1),
                        )
                    nc.scalar.activation(hT[:, m, no:no + nn], h_ps[:, 0:nn], AF.Relu)
                nc.sync.dma_start(out=w2sb[:, m, :], in_=w2src[:, m, :])
                if e + 1 < cfg.E:
                    w1t_n = wp.tile([128, cfg.FT, 128], BF16, tag="w1", bufs=24,
                                    name=f"w1t_n{e}_{m}")
                    nc.scalar.dma_start(out=w1t_n[:], in_=w1_d[e + 1, m])
                    w1_next.append(w1t_n)

            eo = ap_.tile([128, cfg.J, cfg.DO], FP32, tag="eo", bufs=1)
            for j in range(cfg.J):
                o_ps = ps2.tile([128, cfg.DO], FP32, tag="o")
                for h in range(cfg.HT):
                    for (oo, on) in o_chunks:
                        nc.tensor.matmul(
                            o_ps[:, oo:oo + on],
                            hT[:, h, j * 128:(j + 1) * 128],
                            w2sb[:, h, oo:oo + on],
                            start=(h == 0),
                            stop=(h == cfg.HT - 1),
                        )
                nc.scalar.activation(eo[:, j, :], o_ps[:], AF.Exp,
                                     bias=lngb_sb[:, e, j:j + 1])
            if e < cfg.E - 1:
                nc.gpsimd.dma_scatter_add(
                    out_ap=y_ev[:],
                    out_ap_other=y_od[:],
                    in_ap=eo[:],
                    idxs_ap=idxs_sb[e][:],
                    num_idxs=cfg.CAP,
                    num_idxs_reg=cnt_regs[e],
                    elem_size=cfg.DO,
                    sbuf_tokens_per_rank=128,
                    parity_reg=0,
                )
            else:
                # split the last expert's combine per 128-slot tile so the
                # finalize can begin as soon as the final sliver lands
                from concourse.expressions import smax, smin
                for j in range(cfg.J):
                    r_j = smax(smin(cnt_regs[e] - 128 * j, 128), 0)
                    nc.gpsimd.dma_scatter_add(
                        out_ap=y_ev[:],
                        out_ap_other=y_od[:],
                        in_ap=eo[:, j:j + 1, :],
                        idxs_ap=idxs_sb[e][:, j * 8:(j + 1) * 8],
                        num_idxs=128,
                        num_idxs_reg=r_j,
                        elem_size=cfg.DO,
                        sbuf_tokens_per_rank=128,
                        parity_reg=0,
                    )

    # ---------------- finalize: y = log(max(acc, eps-fix)) ----------------
    with tc.tile_pool(name="fin", bufs=3) as fp:
        for i in range(cfg.TT):
            acc = (y_ev if i % 2 == 0 else y_od)[:, i // 2, :]
            yo = fp.tile([128, cfg.DO], FP32, tag="yo")
            nc.scalar.activation(yo[:], acc, AF.Ln, bias=epsb[:])
            nc.sync.dma_start(out=y_d[i * 128:(i + 1) * 128, :], in_=yo[:])

    ctx.close()


# ---------------------------------------------------------------------------
# host side
# ---------------------------------------------------------------------------

_COMPILED = {}


def _get_compiled(cfg: Cfg):
    key = tuple(sorted(cfg.__dict__.items()))
    if key in _COMPILED:
        return _COMPILED[key]
    nc = bacc.Bacc("TRN2", target_bir_lowering=False, debug=False,
                   num_devices=N_CORES, num_swdge_queues=2)
    ins = {
        "xt": nc.dram_tensor("xt", [cfg.D + 16, cfg.BC], FP32, kind="ExternalInput").ap(),
        "xb": nc.dram_tensor("xb", [cfg.BC, cfg.D], BF16, kind="ExternalInput").ap(),
        "wgx": nc.dram_tensor("wgx", [cfg.D, cfg.E], FP32, kind="ExternalInput").ap(),
        "tab16": nc.dram_tensor("tab16", [16, cfg.E], FP32, kind="ExternalInput").ap(),
        "w1": nc.dram_tensor("w1", [cfg.E, cfg.HT, 128, cfg.FT, 128], BF16, kind="ExternalInput").ap(),
        "w2": nc.dram_tensor("w2", [cfg.E, cfg.H, cfg.DO], BF16, kind="ExternalInput").ap(),
    }
    outs = {
        "y": nc.dram_tensor("y", [cfg.BC, cfg.DO], FP32, kind="ExternalOutput").ap(),
        "stats": nc.dram_tensor("stats", [cfg.E, 2], FP32, kind="ExternalOutput").ap(),
    }
    with TileContext(nc) as tc:
        build_moe(tc, outs, ins, cfg)
    nc.compile()
    _COMPILED[key] = nc
    return nc


def _wrap16(ids: np.ndarray) -> np.ndarray:
    """[N] -> [128, N//16] int16, 16-wrapped and replicated across 8 cores."""
    n = ids.shape[0]
    w = ids.reshape(n // 16, 16).T.astype(np.int16)
    return np.tile(w, (8, 1))


def prepare_inputs(x, task_ids, task_matrix, task_W, task_b, w_gate,
                   fc1_w, fc1_b, fc2_w, fc2_b, cfg: Cfg):
    """Shard + lay out inputs for the 8 cores. Pure data marshalling on host
    (plus folding the tiny task-embedding linear algebra into a 10x8 table)."""
    D = cfg.D
    assert np.allclose(fc1_b, 0.0) and np.allclose(fc2_b, 0.0), \
        "kernel assumes zero expert biases (as produced by setup_inputs)"

    # task logit table: Linear(task_emb) @ w_gate[D:]  ->  [16, E] (padded)
    wg_t = w_gate[D:].astype(np.float64)
    tab = (task_matrix.astype(np.float64) @ task_W.astype(np.float64) @ wg_t
           + task_b.astype(np.float64) @ wg_t).astype(np.float32)
    tab16 = np.zeros((16, cfg.E), np.float32)
    tab16[:cfg.NT] = tab

    wgx = np.ascontiguousarray(w_gate[:D], np.float32)
    w1t = np.ascontiguousarray(
        fc1_w.astype(ml_dtypes.bfloat16)
        .reshape(cfg.E, cfg.FT, 128, cfg.HT, 128)
        .transpose(0, 3, 2, 1, 4)
    )
    w2b = np.ascontiguousarray(fc2_w.astype(ml_dtypes.bfloat16))

    in_maps = []
    for c in range(N_CORES):
        xs = x[c * cfg.BC:(c + 1) * cfg.BC]
        ts = task_ids[c * cfg.BC:(c + 1) * cfg.BC]
        onehot = np.zeros((16, cfg.BC), np.float32)
        onehot[ts, np.arange(cfg.BC)] = 1.0
        in_maps.append({
            "xt": np.ascontiguousarray(
                np.concatenate([xs.T.astype(np.float32), onehot], axis=0)),
            "xb": np.ascontiguousarray(xs.astype(ml_dtypes.bfloat16)),
            "wgx": wgx,
            "tab16": tab16,
            "w1": w1t,
            "w2": w2b,
        })
    return in_maps


def _cv_squared(v: np.ndarray) -> np.float32:
    eps = np.float32(1e-10)
    v = v.astype(np.float32)
    return np.var(v, ddof=1, dtype=np.float32) / (np.mean(v, dtype=np.float32) ** 2 + eps)


def kernel(x, task_ids, task_matrix, task_W, task_b, w_gate,
           fc1_w, fc1_b, fc2_w, fc2_b, **run_kwargs):
    cfg = Cfg()
    x = np.asarray(x, np.float32)
    task_ids = np.asarray(task_ids, np.int32)
    in_maps = prepare_inputs(
        x, task_ids, np.asarray(task_matrix, np.float32),
        np.asarray(task_W, np.float32), np.asarray(task_b, np.float32),
        np.asarray(w_gate, np.float32), np.asarray(fc1_w, np.float32),
        np.asarray(fc1_b, np.float32), np.asarray(fc2_w, np.float32),
        np.asarray(fc2_b, np.float32), cfg)
    nc = _get_compiled(cfg)
    res = run_bass_kernel_spmd(nc, in_maps, list(range(N_CORES)), **run_kwargs)
    y = np.concatenate([res.results[c]["y"] for c in range(N_CORES)], axis=0)
    stats = np.stack([res.results[c]["stats"] for c in range(N_CORES)])
    importance = stats[:, :, 0].sum(0)
    load = stats[:, :, 1].sum(0)
    loss = np.float32(_cv_squared(importance) + _cv_squared(load))
    kernel.last_results = res
    return y, loss
